# revision 18
# baseline (speedup 1.0000x reference)
"""Multi-head attention (B=4, S=2048, E=1024, H=16) on 8 TRN2 NeuronCores.

Sharding: batch x head-group tensor parallel -- core c = 2*b + hg handles
batch b and heads hg*8 .. hg*8+7 for ALL 2048 queries.  Q/K/V projections
are column-split by head (each core projects only its 8 heads); the output
projection is row-split (each core contracts its 512 E-rows of W_out) and
produces a partial [E, S] output that the HOST sums across the core pair
while unsharding (the "all-reduce" of the sharding hint, done on host).

Per-core kernel:
  - Q^T/K^T projections (bf16 matmul, fp32 PSUM) evacuated with fused
    bias-add + fp8e4 quantization (DVE tensor_scalar_add, fp8 out).
  - scores via fp8 DoubleRow matmuls: contraction d=64 fed as
    [64 part, 2(dup, stride 0), N]; the duplicated group doubles the
    result and the exp activation scale absorbs the factor 2.
    Cost: 0.5 cycles/row (vs 1.0 bf16).
  - exp on ScalarE (the bottleneck engine, ~266us busy): one [128, 1024]
    activation per j-PAIR, read from a single manual 4-slot PSUM ring
    [128, 4, 512] (slot = j%4; pairs are slot-aligned so the read AP is
    flat, and the two pairs double-buffer against each other).
  - PV in the FLIPPED orientation: out[q=128, 65] = P_tile.T @ [V | ones]
    (all 128 output partitions vs 65 in the naive orientation); the ones
    column gives the softmax denominator per query row.  PSUM start=True
    zeroes a whole 2KB region, so the four per-qt accumulators live in a
    DVE-zeroed [128, 4, 128] tile (qt stride 512B -> no bank crossing)
    and accumulate with start=False.
  - normalization: per-partition reciprocal of the denominator column +
    tensor_scalar multiply during evacuation -> O in [q, d] layout,
    PE-transposed (identity matmul) back to O^T for the out projection.

Schedule: 4 query-quarter phase groups x 8 heads x 16 key tiles.
Projections (K/Q half-chunks, per-(head, key-tile) V chunks), transposes
and the PREVIOUS quarter's output projection are interleaved one small
chunk per j-step into the PE stream so the exp engine never starves;
only the last quarter's output projection is a serial tail.  Split
chunks sharing one PSUM tile are kept adjacent (the 2-buffer proj pool
tolerates at most one intervening allocation).
"""

import sys

if "/opt/trn_rl_repo" not in sys.path:
    sys.path.insert(0, "/opt/trn_rl_repo")

import numpy as np
import ml_dtypes

B, S, E, H = 4, 2048, 1024, 16
P = 128
HD = 64           # head dim
NH = 8            # heads per core
DT = 4            # d-tiles (head pairs) per core
ET = E // P       # 8 e-tiles (contraction for projections)
ST = S // P       # 16 key tiles
N_CORES = 8
QQ = 512          # query quarter width
NQ = S // QQ      # 4 quarters
SCALE = 1.0 / float(np.sqrt(HD))

_BF16 = ml_dtypes.bfloat16

_cached = None


def _build():
    import concourse.bass as bass
    import concourse.tile as tile
    import concourse.mybir as mybir
    from concourse import bacc

    dt = mybir.dt
    nc = bacc.Bacc("TRN2", target_bir_lowering=False, debug=False)

    xt_d = nc.dram_tensor("xt", [E, S], dt.bfloat16, kind="ExternalInput").ap()
    wq_d = nc.dram_tensor("wq", [E, 512], dt.bfloat16, kind="ExternalInput").ap()
    wk_d = nc.dram_tensor("wk", [E, 512], dt.bfloat16, kind="ExternalInput").ap()
    wv_d = nc.dram_tensor("wv", [E, 512], dt.bfloat16, kind="ExternalInput").ap()
    wo_d = nc.dram_tensor("wo", [512, E], dt.bfloat16, kind="ExternalInput").ap()
    bq_d = nc.dram_tensor("bq", [P, DT], dt.float32, kind="ExternalInput").ap()
    bk_d = nc.dram_tensor("bk", [P, DT], dt.float32, kind="ExternalInput").ap()
    bv_d = nc.dram_tensor("bv", [1, 512], dt.bfloat16, kind="ExternalInput").ap()
    bo_d = nc.dram_tensor("bo", [P, ET], dt.float32, kind="ExternalInput").ap()
    iden_d = nc.dram_tensor("iden", [P, P], dt.bfloat16, kind="ExternalInput").ap()
    out_d = nc.dram_tensor("out", [E, S], dt.float32, kind="ExternalOutput").ap()

    DR = mybir.MatmulPerfMode.DoubleRow

    with tile.TileContext(nc) as tc:
        with (
            tc.tile_pool(name="const", bufs=1) as cpool,
            tc.tile_pool(name="acts", bufs=1) as apool,
            tc.tile_pool(name="pp", bufs=3) as ppool,        # P (exp out)
            tc.tile_pool(name="oqp", bufs=2) as oqpool,      # O [q, dd] staging
            tc.tile_pool(name="recp", bufs=2) as recpool,    # reciprocals
            tc.tile_pool(name="outs", bufs=4) as outpool,    # out staging
            tc.tile_pool(name="pssc", bufs=1, space="PSUM") as scpool,   # 4 banks
            tc.tile_pool(name="pspv", bufs=1, space="PSUM") as pvpool,   # 1 bank
            tc.tile_pool(name="pspj", bufs=2, space="PSUM") as pjpool,   # 2 banks
            tc.tile_pool(name="pstp", bufs=1, space="PSUM") as tppool,   # 1 bank
        ):
            # ---------------- constants / inputs -----------------------
            xt = cpool.tile([P, ET, S], dt.bfloat16)
            wq = cpool.tile([P, ET, 512], dt.bfloat16)
            wk = cpool.tile([P, ET, 512], dt.bfloat16)
            wv = cpool.tile([P, ET, 512], dt.bfloat16)
            wo = cpool.tile([P, DT, E], dt.bfloat16)
            bq = cpool.tile([P, DT], dt.float32)
            bk = cpool.tile([P, DT], dt.float32)
            bv = cpool.tile([1, 512], dt.bfloat16)
            bo = cpool.tile([P, ET], dt.float32)
            iden = cpool.tile([P, P], dt.bfloat16)
            ones1 = cpool.tile([1, P], dt.bfloat16)

            # activations
            qt8 = apool.tile([P, DT, S], dt.float8e4)   # Q^T (bias+fp8)
            kt8 = apool.tile([P, DT, S], dt.float8e4)   # K^T (bias+fp8)
            va = apool.tile([P, ST, NH, HD + 1], dt.bfloat16)  # V | ones
            scb = apool.tile([P, DT, S], dt.bfloat16)   # O^T (normalized)

            # single long-lived PSUM tiles (region-level dependencies)
            ring = scpool.tile([P, 4, 512], dt.float32, tag="sc", name="ring")
            pv = pvpool.tile([P, 4, P], dt.float32, tag="pv", name="pv")

            nc.sync.dma_start(wk[:, :, :], wk_d.rearrange("(eo p) c -> p eo c", p=P))
            for e in range(ET):
                nc.sync.dma_start(xt[:, e, :], xt_d[e * P : (e + 1) * P, :])
            nc.sync.dma_start(wq[:, :, :], wq_d.rearrange("(eo p) c -> p eo c", p=P))
            nc.sync.dma_start(wv[:, :, :], wv_d.rearrange("(eo p) c -> p eo c", p=P))
            nc.sync.dma_start(bq[:], bq_d)
            nc.sync.dma_start(bk[:], bk_d)
            nc.sync.dma_start(bv[:], bv_d)
            nc.sync.dma_start(iden[:], iden_d)
            nc.sync.dma_start(wo[:], wo_d.rearrange("(eo p) c -> p eo c", p=P))
            nc.sync.dma_start(bo[:], bo_d)
            nc.gpsimd.memset(ones1[:], 1.0)
            nc.gpsimd.memset(va[:, :, :, HD : HD + 1], 1.0)

            # ---------------- small-chunk emitters ----------------------
            # Pieces are kept under ~1us of PE time so they interleave into
            # attention j-steps without starving the exp engine.  kq/outproj
            # chunks come as (partA, partB) sharing one psum tile; both
            # halves must be emitted with at most one pj alloc in between.

            def kq_halves(which, t, c):
                w_, b_, dst = (wk, bk, kt8) if which == "k" else (wq, bq, qt8)
                holder = {}

                def part0():
                    ps = pjpool.tile(
                        [P, 512], dt.float32, tag="pj", name=f"{which}{t}{c}"
                    )
                    holder[0] = ps
                    for e in range(4):
                        nc.tensor.matmul(
                            ps[:],
                            w_[:, e, t * P : (t + 1) * P],
                            xt[:, e, c * 512 : (c + 1) * 512],
                            start=(e == 0),
                            stop=False,
                        )

                def part1():
                    ps = holder[0]
                    for e in range(4, ET):
                        nc.tensor.matmul(
                            ps[:],
                            w_[:, e, t * P : (t + 1) * P],
                            xt[:, e, c * 512 : (c + 1) * 512],
                            start=False,
                            stop=(e == ET - 1),
                        )
                    nc.vector.tensor_scalar_add(
                        dst[:, t, c * 512 : (c + 1) * 512], ps[:], b_[:, t : t + 1]
                    )

                return [part0, part1]

            def v_chunk(h, st):
                """V rows for (head h, key tile st): [128 keys, 64] + bias."""
                def go():
                    ps = pjpool.tile([P, 512], dt.float32, tag="pj", name=f"v{h}{st}")
                    for e in range(ET):
                        nc.tensor.matmul(
                            ps[:, 0:HD],
                            xt[:, e, st * P : (st + 1) * P],
                            wv[:, e, h * HD : (h + 1) * HD],
                            start=(e == 0),
                            stop=False,
                        )
                    nc.tensor.matmul(
                        ps[:, 0:HD],
                        ones1[0:1, :],
                        bv[0:1, h * HD : (h + 1) * HD],
                        start=False,
                        stop=True,
                    )
                    nc.vector.tensor_copy(va[:, st, h, 0:HD], ps[:, 0:HD])
                return go

            def outproj_halves(eo, qq):
                holder = {}
                q0 = qq * QQ

                def part0():
                    ps = pjpool.tile([P, 512], dt.float32, tag="pj", name=f"o{eo}{qq}")
                    holder[0] = ps
                    for t in (0, 1):
                        nc.tensor.matmul(
                            ps[:],
                            wo[:, t, eo * P : (eo + 1) * P],
                            scb[:, t, q0 : q0 + 512],
                            start=(t == 0),
                            stop=False,
                        )

                def part1():
                    ps = holder[0]
                    for t in (2, 3):
                        nc.tensor.matmul(
                            ps[:],
                            wo[:, t, eo * P : (eo + 1) * P],
                            scb[:, t, q0 : q0 + 512],
                            start=False,
                            stop=(t == DT - 1),
                        )
                    ot = outpool.tile(
                        [P, 512], dt.float32, tag="ot", name=f"oe{eo}{qq}"
                    )
                    nc.vector.tensor_scalar_add(ot[:], ps[:], bo[:, eo : eo + 1])
                    nc.sync.dma_start(
                        out_d[eo * P : (eo + 1) * P, q0 : q0 + 512], ot[:]
                    )

                return [part0, part1]

            oq_tiles = {}

            def transpose_item(t, qq, qt):
                def go():
                    tp = tppool.tile(
                        [P, P], dt.bfloat16, tag="tp", name=f"tp{t}{qq}{qt}"
                    )
                    nc.tensor.transpose(tp[:], oq_tiles[(t, qq)][:, qt, :], iden[:])
                    q0 = qq * QQ + qt * P
                    nc.vector.tensor_copy(scb[:, t, q0 : q0 + P], tp[:])
                return go

            # ---------------- attention phase ---------------------------
            def emit_attention(h, qq, extra):
                t, hp = h // 2, (h % 2) * HD
                nc.vector.memset(pv[:], 0.0)
                q0 = qq * QQ
                for j in range(ST):
                    s = j % 4
                    nc.tensor.matmul(
                        ring[:, s, :],
                        kt8[hp : hp + HD, t, j * P : (j + 1) * P]
                        .unsqueeze(1)
                        .broadcast_to((HD, 2, P)),
                        qt8[hp : hp + HD, t, q0 : q0 + 512]
                        .unsqueeze(1)
                        .broadcast_to((HD, 2, 512)),
                        start=True,
                        stop=True,
                        perf_mode=DR,
                    )
                    if j % 2 == 1:
                        # exp over the aligned slot pair (s-1, s) -> [128,1024]
                        ptile = ppool.tile(
                            [P, 2, 512], dt.bfloat16, tag="p", name=f"p{h}{qq}{j}"
                        )
                        nc.scalar.activation(
                            ptile.rearrange("p a b -> p (a b)"),
                            ring[:, s - 1 : s + 1, :].rearrange("p a b -> p (a b)"),
                            mybir.ActivationFunctionType.Exp,
                            scale=SCALE / 2.0,
                        )
                        for jj in (j - 1, j):
                            for qt in range(4):
                                nc.tensor.matmul(
                                    pv[:, qt, 0 : HD + 1],
                                    ptile[:, jj % 2, qt * P : (qt + 1) * P],
                                    va[:, jj, h, :],
                                    start=False,
                                    stop=(jj == ST - 1),
                                    skip_group_check=True,
                                )
                    if extra:
                        extra.pop(0)()

            def emit_evac(h, qq):
                t, half = h // 2, h % 2
                if half == 0:
                    oq_tiles[(t, qq)] = oqpool.tile(
                        [P, 4, P], dt.bfloat16, tag="oq", name=f"oq{t}{qq}"
                    )
                oq = oq_tiles[(t, qq)]
                rec = recpool.tile([P, 4], dt.float32, tag="rec", name=f"rc{h}{qq}")
                scr = recpool.tile([P, 4], dt.float32, tag="scr", name=f"sr{h}{qq}")
                nc.vector.reciprocal_approx_accurate(
                    rec[:], pv[:, :, HD : HD + 1].rearrange("p a b -> p (a b)"), scr[:]
                )
                for qt in range(4):
                    nc.vector.tensor_scalar(
                        oq[:, qt, half * HD : (half + 1) * HD],
                        pv[:, qt, 0:HD],
                        rec[:, qt : qt + 1],
                        None,
                        op0=mybir.AluOpType.mult,
                    )

            # ---------------- emission schedule -------------------------
            # Pre-attention prefix: K(dt0, kc0/kc1), Q(dt0, qq0), V(h0, st0..5)
            for fn in kq_halves("k", 0, 0) + kq_halves("q", 0, 0) + kq_halves("k", 0, 1):
                fn()
            for st in range(6):
                v_chunk(0, st)()

            # Per-phase thunk lists.  Slot i is consumed at the end of j-step
            # i, so an item at slot i is available from step i+1 onward.
            # V(h, st) is needed by the PV matmul at step st (its own phase),
            # so it must sit at slot <= st-1; the due-critical V items are
            # placed first and everything else is appended after them.
            plans = {(h, qq): [] for qq in range(NQ) for h in range(NH)}
            plans[(0, 0)] = (
                [v_chunk(0, 6), v_chunk(0, 7)]
                + kq_halves("k", 0, 2)
                + [v_chunk(0, 8), v_chunk(0, 9)]
                + kq_halves("k", 0, 3)
                + [v_chunk(0, st) for st in range(10, ST)]
            )
            # Each head's V just-in-time in its own qq0 phase (slot st-2);
            # its first two tiles at the end of the previous head's phase.
            for h in range(1, NH):
                plans[(h - 1, 0)].extend([v_chunk(h, 0), v_chunk(h, 1)])
                plans[(h, 0)] = [v_chunk(h, st) for st in range(2, ST)] + plans[(h, 0)]
            # K for d-tiles 1..3 (due at head 2t, quarter 0) and Q(dt, qq0):
            # appended to the two phases before head 2t.
            for t2 in range(1, DT):
                for c in range(4):
                    plans[(2 * t2 - 2 + (c % 2), 0)].extend(kq_halves("k", t2, c))
                plans[(2 * t2 - 1, 0)].extend(kq_halves("q", t2, 0))
            # Q for quarters 1..3: dt0 just before the quarter starts; dt1..3
            # inside the quarter, before head 2*dt reaches it.
            for qc in range(1, NQ):
                plans[(NH - 2, qc - 1)].extend(kq_halves("q", 0, qc))
                for t2 in range(1, DT):
                    plans[(min(t2, 2 * t2 - 1), qc)].extend(kq_halves("q", t2, qc))
            # outproj of quarter qq-1 runs during quarter qq (heads 1..4)
            for qq in range(1, NQ):
                for eo in range(ET):
                    plans[(1 + (eo % 4), qq)].extend(outproj_halves(eo, qq - 1))
            # transposes: all pairs of quarter qq -> phase (h0, qq+1), which
            # is otherwise empty (they are only needed by outproj(qq), which
            # starts at (h1, qq+1)).  Final quarter: spread over the odd
            # phases after each pair completes; pair 3 goes to the tail.
            order = [(h, qq) for qq in range(NQ) for h in range(NH)]
            tail_items = []
            for qq in range(NQ):
                for t2 in range(DT):
                    items = [transpose_item(t2, qq, qt) for qt in range(4)]
                    if qq < NQ - 1:
                        plans[(0, qq + 1)].extend(items)
                    elif t2 < DT - 1:
                        plans[(2 * t2 + 3, qq)].extend(items)
                    else:
                        tail_items.extend(items)

            # Run all phases.
            for h, qq in order:
                emit_attention(h, qq, plans[(h, qq)])
                emit_evac(h, qq)
                for it in plans[(h, qq)]:
                    it()
                plans[(h, qq)] = []

            # Tail: last pair's transposes + final quarter's outproj.
            for it in tail_items:
                it()
            for eo in range(ET):
                for fn in outproj_halves(eo, NQ - 1):
                    fn()

    nc.compile()
    return nc


def _prep_inputs(x, W_qkv, b_qkv, W_out, b_out):
    """Host-side sharding + layout prep. Returns per-core input maps."""
    w = W_qkv.reshape(E, H, 3, HD)
    b3 = b_qkv.reshape(H, 3, HD)
    iden = np.eye(P, dtype=np.float32).astype(_BF16)

    in_maps = []
    for core in range(N_CORES):
        b, hg = core // 2, core % 2
        hs = slice(hg * NH, (hg + 1) * NH)
        xt = np.ascontiguousarray(x[b].T).astype(_BF16)           # [E, S]
        wq = np.ascontiguousarray(w[:, hs, 0, :].reshape(E, 512)).astype(_BF16)
        wk = np.ascontiguousarray(w[:, hs, 1, :].reshape(E, 512)).astype(_BF16)
        wv = np.ascontiguousarray(w[:, hs, 2, :].reshape(E, 512)).astype(_BF16)
        wo = np.ascontiguousarray(W_out[hg * 512 : (hg + 1) * 512, :]).astype(_BF16)
        bq = np.ascontiguousarray(b3[hs, 0, :].reshape(DT, P).T).astype(np.float32)
        bk = np.ascontiguousarray(b3[hs, 1, :].reshape(DT, P).T).astype(np.float32)
        bv = np.ascontiguousarray(b3[hs, 2, :].reshape(1, 512)).astype(_BF16)
        bo = (np.ascontiguousarray(b_out.reshape(ET, P).T) * (1.0 if hg == 0 else 0.0)).astype(np.float32)
        in_maps.append(
            {
                "xt": xt,
                "wq": wq,
                "wk": wk,
                "wv": wv,
                "wo": wo,
                "bq": bq,
                "bk": bk,
                "bv": bv,
                "bo": bo,
                "iden": iden,
            }
        )
    return in_maps


def run_raw(x, W_qkv, b_qkv, W_out, b_out, trace=False, **kw):
    """Run on hardware; returns (full_output [B,S,E] f32, BassKernelResults)."""
    global _cached
    from concourse.bass_utils import run_bass_kernel_spmd

    if _cached is None:
        _cached = _build()
    nc = _cached
    in_maps = _prep_inputs(
        np.asarray(x), np.asarray(W_qkv), np.asarray(b_qkv),
        np.asarray(W_out), np.asarray(b_out),
    )
    res = run_bass_kernel_spmd(
        nc, in_maps, core_ids=list(range(N_CORES)), trace=trace, **kw
    )
    out = np.empty((B, S, E), dtype=np.float32)
    for b in range(B):
        acc = np.asarray(res.results[2 * b]["out"]) + np.asarray(
            res.results[2 * b + 1]["out"]
        )
        out[b] = acc.T
    return out, res


def kernel(x, W_qkv, b_qkv, W_out, b_out):
    out, _ = run_raw(x, W_qkv, b_qkv, W_out, b_out, trace=False)
    return out
